# revision 28
# baseline (speedup 1.0000x reference)
"""Multi-head attention (B=4, N=2048, C=768, H=12, Dh=64) on 8 TRN2 NeuronCores.

Sharding: tensor-parallel on heads. 2 cores per batch; each core owns 6 of the
12 heads over the FULL 2048-token sequence, so no K/V projection work is
duplicated (a query-split layout computes each batch's K/V twice; this layout
cuts per-core PE work from 614k to 541k cycles). Each core emits a partial
projection output [2048, 768] (its heads' slice of the contraction); the host
sums the two partials per batch and adds the bias as the unshard step (the
spec's "all-reduce after proj" done host-side).

Per-core inputs (partition dim first):
  xT     [768, 2048]  bf16  x[b].T
  wqkvT  [768, 1152]  bf16  [q | k | v] columns for this core's 6 heads
  wprojT [384, 768]   bf16  proj_w.T rows for this core's 6 heads
  out    [2048, 768]  f32   partial projection output

Schedule note — the chip power limiter is the binding constraint: all 8 cores
share one NeuronDevice's power envelope, and firmware clamps the PE to K=4/8
(1.2 GHz) when sustained PE duty is too high. Variants that overlap qkv into
the attention flow (raising attention-phase PE duty from ~80% toward ~95%)
measured 373->477/412/373 us with 120-180 us spent clamped. The schedule here
(qkv burst upfront, then ScalarE-exp-paced attention at ~80% PE duty)
measures fastest (318 us, throttle ~13-23 us). Attention is ScalarE-paced:
exp of all 25.2M logits at 153.6 G/s is ~170 us min; PE stream is 225 us.

Pipeline:
  - PE warmup matmuls bridge the initial DMA window so the HAM clock gate is
    warm when the real qkv stream starts.
  - qkv projection in bf16 (psum f32), all upfront; q/k stored transposed
    [d, n] packed two heads per 128-partition tile; v stored [token, d]
    augmented with a ones column per head ([64 v | 1] x 6 heads).
  - scores: S^T chunks [128 kv-rows, 512 q] = kT-slice.T @ qT-slice (K=64);
    exp on ScalarE over [128, <=1536] PSUM spans with the 1/sqrt(Dh) scale
    folded into the ACTIVATE affine (no max-subtraction; |S|<=~10 here).
  - PV: one matmul per (kv-chunk, head) with augmented V stationary [128, 65]
    -> attn-out.T rows 0..63 + softmax denominator at row 64 of the same PSUM
    accumulation group.
  - the two heads of a pair run as separate score/exp/PV streams, so each
    head's normalize chain (DVE reciprocal + gpsimd partition-broadcast +
    DVE multiply -> bf16 attnT tiles) overlaps the other head's stream
    instead of stalling the next unit's PV accumulation.
  - projection of query block qb runs inside units (qb+1, hp0/hp1), 2 row
    blocks each: the unit emits its h2=0 scores+exps first, then the proj
    matmuls, then the deferred PVs, so ScalarE chews the queued exp backlog
    instead of idling while the PE projects. Last block's projection tails.
  PSUM: st-tag 2x3 banks (shared by qkv groups, scores, proj) + pv 2x1 = 8.
"""

import sys

if "/opt/trn_rl_repo" not in sys.path:
    sys.path.insert(0, "/opt/trn_rl_repo")

import numpy as np
import ml_dtypes

B, N, C = 4, 2048, 768
H, Dh = 12, 64
HC = H // 2            # heads per core
HPC = HC // 2          # head-pairs per core (3)
CW = HC * Dh           # 384 output-d columns per core
SCALE = Dh ** -0.5
CCH = C // 128         # 6 contraction chunks
NCORES = 8
JG = [(0, 3), (3, 3), (6, 3), (9, 3), (12, 2), (14, 2)]  # kv j-block groups

_NC_CACHE = {}


def _build():
    import concourse.bass as bass
    import concourse.tile as tile
    import concourse.mybir as mybir
    from concourse import bacc

    f32 = mybir.dt.float32
    bf16 = mybir.dt.bfloat16
    Exp = mybir.ActivationFunctionType.Exp

    nc = bacc.Bacc(
        "TRN2",
        target_bir_lowering=False,
        debug=False,
        enable_asserts=False,
        num_devices=NCORES,
    )

    xT = nc.dram_tensor("xT", [C, N], bf16, kind="ExternalInput").ap()
    wqkvT = nc.dram_tensor("wqkvT", [C, 3 * CW], bf16, kind="ExternalInput").ap()
    wprojT = nc.dram_tensor("wprojT", [CW, C], bf16, kind="ExternalInput").ap()
    out = nc.dram_tensor("out", [N, C], f32, kind="ExternalOutput").ap()

    with tile.TileContext(nc) as tc:
        from contextlib import ExitStack

        with ExitStack() as ctx:
            singles = ctx.enter_context(tc.tile_pool(name="singles", bufs=1))
            psum = ctx.enter_context(tc.tile_pool(name="psum", bufs=1, space="PSUM"))

            # ---- load phase-A inputs (released after qkv) ---------------
            load = tc.alloc_tile_pool(name="load", bufs=1)
            xt = [load.tile([128, N], bf16, tag=f"xt{i}", name=f"xt{i}")
                  for i in range(CCH)]
            wq = [load.tile([128, 3 * CW], bf16, tag=f"wq{i}", name=f"wq{i}")
                  for i in range(CCH)]
            for i in range(CCH):
                nc.gpsimd.dma_start(out=wq[i][:, 0:128],
                                    in_=wqkvT[i * 128:(i + 1) * 128, 0:128])
            for i in range(CCH):
                nc.gpsimd.dma_start(out=wq[i][:, 128:],
                                    in_=wqkvT[i * 128:(i + 1) * 128, 128:])
            for nch in range(N // 512):
                for i in range(CCH):
                    nc.sync.dma_start(
                        out=xt[i][:, nch * 512:(nch + 1) * 512],
                        in_=xT[i * 128:(i + 1) * 128, nch * 512:(nch + 1) * 512])
            wp = []
            for i in range(HPC):
                t = singles.tile([128, C], bf16, tag=f"wp{i}", name=f"wp{i}")
                nc.gpsimd.dma_start(out=t, in_=wprojT[i * 128:(i + 1) * 128, :])
                wp.append(t)

            # ---- PE warmup: HAM un-throttles after ~3.4 us of activity;
            # dummy matmuls on a memset tile (no DMA dependency) keep the
            # clock gate warm across the DMA window so the real qkv stream
            # starts at 2.4 GHz ------------------------------------------
            warm = singles.tile([128, 512], bf16, tag="warm", name="warm")
            nc.vector.memset(warm, 0.0)
            for _ in range(30):
                wps = psum.tile([128, 512], f32, tag="pv", bufs=2,
                                name="warmps")
                nc.tensor.matmul(wps, lhsT=warm[:, 0:128], rhs=warm,
                                 start=True, stop=True)

            # ---- qkv projections, results stored bf16 -------------------
            qt = [singles.tile([128, N], bf16, tag=f"qt{i}", name=f"qt{i}")
                  for i in range(HPC)]
            kt = [singles.tile([128, N], bf16, tag=f"kt{i}", name=f"kt{i}")
                  for i in range(HPC)]
            # v_aug: per 128-token tile, 6 heads x (64 v-cols + ones col)
            vt = [singles.tile([128, HC * 65], bf16, tag=f"vt{i}", name=f"vt{i}")
                  for i in range(N // 128)]

            # qT[d, n] and kT[d, n], two heads per 128-partition tile.
            # nch-outer so group order matches the xT chunk DMA arrival
            # order: the PE consumes ~7.7 us of groups per chunk while the
            # DMA delivers one every ~3.3 us, so it never stalls mid-phase
            # (a dt-major order stalled on chunk 3 and drew an HAM
            # re-throttle inside the qkv burst).
            for nch in range(N // 512):
                for dt in range(HPC):
                    for (dst, col0) in ((qt[dt], dt * 128),
                                        (kt[dt], CW + dt * 128)):
                        ps = psum.tile([128, 512], f32, tag="st", bufs=2,
                                       name="ps_qk")
                        for cc in range(CCH):
                            nc.tensor.matmul(
                                ps,
                                lhsT=wq[cc][:, col0:col0 + 128],
                                rhs=xt[cc][:, nch * 512:(nch + 1) * 512],
                                start=(cc == 0), stop=(cc == CCH - 1),
                            )
                        nc.vector.tensor_copy(
                            dst[:, nch * 512:(nch + 1) * 512], ps)

            # v in [token, d] layout: v[n, d] = sum_c xT[c, n] * wv[c, d]
            for nt in range(N // 128):
                vaug = vt[nt].rearrange("p (h e) -> p h e", e=65)
                nc.vector.memset(vaug[:, :, 64:65], 1.0)
                ps = psum.tile([128, 512], f32, tag="st", bufs=2, name="ps_v")
                for cc in range(CCH):
                    nc.tensor.matmul(
                        ps[:, :CW],
                        lhsT=xt[cc][:, nt * 128:(nt + 1) * 128],
                        rhs=wq[cc][:, 2 * CW:3 * CW],
                        start=(cc == 0), stop=(cc == CCH - 1),
                    )
                nc.vector.tensor_copy(
                    vaug[:, :, 0:64],
                    ps[:, :CW].rearrange("p (h e) -> p h e", e=64),
                )

            load.release()

            # ---- attention ----------------------------------------------
            work = ctx.enter_context(tc.tile_pool(name="work", bufs=4))
            att = [singles.tile([128, N], bf16, tag=f"att{i}", name=f"att{i}")
                   for i in range(HPC)]

            def proj_block(ic):
                """Projection for one 128-row block (partial: this core's d)."""
                pj = psum.tile([128, C], f32, tag="st", bufs=2, name="pj")
                # dt outer: the dt<2 matmuls depend only on already-normalized
                # head-pairs, so the tail blocks overlap the last unit's
                # normalize chain instead of queuing behind the dt=2 wait
                for dt in range(HPC):
                    for (d0, dw) in ((0, 512), (512, 256)):
                        nc.tensor.matmul(
                            pj[:, d0:d0 + dw],
                            lhsT=att[dt][:, ic * 128:(ic + 1) * 128],
                            rhs=wp[dt][:, d0:d0 + dw],
                            start=(dt == 0), stop=(dt == HPC - 1),
                        )
                osb = work.tile([128, C], f32, tag="osb", bufs=3, name="osb")
                nc.vector.tensor_copy(osb, pj)
                nc.sync.dma_start(out=out[ic * 128:(ic + 1) * 128, :], in_=osb)

            def attn_unit(qb, hp, proj_blocks=()):
                """Scores+exp+PV+normalize, one 512-q block x head pair.

                The two heads run as separate inline streams (each head's
                normalize overlaps the other head's stream). Projection row
                blocks are inserted ONE at a time mid-stream: ScalarE's exp
                run-ahead is capped at ~2 st-spans (~3.1 us) by the 2-buffer
                PSUM rotation, so a single 2.6-us proj block hides under the
                queued exps where a multi-block window cannot.
                """
                pv = [psum.tile([128, 512], f32, tag="pv", bufs=2,
                                name=f"pv{h2}") for h2 in range(2)]
                pb = list(proj_blocks)
                for h2 in range(2):
                    hb = h2 * 64
                    for gi, (j0, jn) in enumerate(JG):
                        st = psum.tile([128, 1536], f32, tag="st", bufs=2,
                                       name="st")
                        for cx in range(jn):
                            j = j0 + cx
                            nc.tensor.matmul(
                                st[:, cx * 512:(cx + 1) * 512],
                                lhsT=kt[hp][hb:hb + 64, j * 128:(j + 1) * 128],
                                rhs=qt[hp][hb:hb + 64, qb * 512:(qb + 1) * 512],
                                start=True, stop=True,
                            )
                        et = work.tile([128, 1536], bf16, tag="et", bufs=10,
                                       name="et")
                        nc.scalar.activation(et[:, :jn * 512], st[:, :jn * 512],
                                             Exp, scale=SCALE)
                        for cx in range(jn):
                            j = j0 + cx
                            h = hp * 2 + h2
                            nc.tensor.matmul(
                                pv[h2][0:65, :],
                                lhsT=vt[j][:, h * 65:(h + 1) * 65],
                                rhs=et[:, cx * 512:(cx + 1) * 512],
                                start=(j == 0), stop=(j == N // 128 - 1),
                            )
                        if gi == 2 and pb:
                            proj_block(pb.pop(0))
                    srow = work.tile([1, 512], f32, tag="srow", bufs=4,
                                     name="srow")
                    nc.vector.tensor_copy(srow, pv[h2][64:65, :])
                    sinv = work.tile([1, 512], f32, tag="sinv", bufs=4,
                                     name="sinv")
                    nc.vector.reciprocal_approx_fast(sinv, srow)
                    bc = work.tile([64, 512], f32, tag="bc", bufs=4, name="bc")
                    nc.gpsimd.partition_broadcast(bc, sinv)
                    nc.vector.tensor_mul(
                        att[hp][h2 * 64:h2 * 64 + 64, qb * 512:(qb + 1) * 512],
                        pv[h2][0:64, :],
                        bc,
                    )
                for ic in pb:
                    proj_block(ic)

            for qb in range(N // 512):           # 512-wide query block
                for hp in range(HPC):            # head pair
                    if qb > 0 and hp < 2:
                        pblks = range((qb - 1) * 4 + 2 * hp,
                                      (qb - 1) * 4 + 2 * hp + 2)
                    else:
                        pblks = ()
                    attn_unit(qb, hp, proj_blocks=pblks)
            for ic in range(12, 16):
                proj_block(ic)

    nc.compile()
    return nc


def _get_nc():
    if "nc" not in _NC_CACHE:
        _NC_CACHE["nc"] = _build()
    return _NC_CACHE["nc"]


def _ensure_ntff_hook():
    """The agent image's ``antenv`` lacks ``axon_hooks``; synthesize it so
    ``run_bass_kernel_spmd(trace=True)`` can capture NTFF profiles."""
    import types
    try:
        from antenv.axon_hooks import get_axon_ntff_profile_hook  # noqa: F401
        return
    except ImportError:
        pass
    import antenv
    from trn_agent_boot.trn_boot import _ntff_profile_via_ctypes
    hook = _ntff_profile_via_ctypes("/opt/axon/libaxon_pjrt.so")
    mod = types.ModuleType("antenv.axon_hooks")
    mod._hook = hook
    mod.get_axon_ntff_profile_hook = lambda: mod._hook

    def _set(h):
        mod._hook = h

    mod.set_axon_ntff_profile_hook = _set
    sys.modules["antenv.axon_hooks"] = mod
    antenv.axon_hooks = mod


def kernel(trace=False, **inputs):
    x = np.asarray(inputs["x"], np.float32)
    qkv_w = np.asarray(inputs["qkv_w"], np.float32)
    proj_w = np.asarray(inputs["proj_w"], np.float32)
    proj_b = np.asarray(inputs["proj_b"], np.float32)

    nc = _get_nc()

    xTb = np.ascontiguousarray(x.transpose(0, 2, 1)).astype(ml_dtypes.bfloat16)
    wqkvT = np.ascontiguousarray(qkv_w.T).astype(ml_dtypes.bfloat16)
    wprojT = np.ascontiguousarray(proj_w.T).astype(ml_dtypes.bfloat16)

    in_maps = []
    for c in range(NCORES):
        b, hg = divmod(c, 2)
        cs = hg * CW
        wslice = np.concatenate(
            [wqkvT[:, s + cs:s + cs + CW] for s in (0, C, 2 * C)], axis=1)
        in_maps.append({
            "xT": xTb[b],
            "wqkvT": np.ascontiguousarray(wslice),
            "wprojT": np.ascontiguousarray(wprojT[cs:cs + CW, :]),
        })

    from concourse import bass_utils
    if trace:
        _ensure_ntff_hook()
        bass_utils.upload_artifacts = lambda tmpdir: tmpdir
    res = bass_utils.run_bass_kernel_spmd(
        nc, in_maps, core_ids=list(range(NCORES)), trace=trace,
    )

    out = np.empty((B, N, C), np.float32)
    for b in range(B):
        out[b] = res.results[2 * b]["out"] + res.results[2 * b + 1]["out"]
    out += proj_b

    if trace:
        return out, res
    return out


# revision 30
# speedup vs baseline: 1.1069x; 1.1069x over previous
"""Multi-head attention (B=4, N=2048, C=768, H=12, Dh=64) on 8 TRN2 NeuronCores.

Sharding: tensor-parallel on heads. 2 cores per batch; each core owns 6 of the
12 heads over the FULL 2048-token sequence, so no K/V projection work is
duplicated (a query-split layout computes each batch's K/V twice; this layout
cuts per-core PE work from 614k to 541k cycles). Each core emits a partial
projection output [2048, 768] (its heads' slice of the contraction); the host
sums the two partials per batch and adds the bias as the unshard step (the
spec's "all-reduce after proj" done host-side).

Per-core inputs (partition dim first):
  xT     [768, 2048]  bf16  x[b].T
  wqkvT  [768, 1152]  bf16  [q | k | v] columns for this core's 6 heads
  wprojT [384, 768]   bf16  proj_w.T rows for this core's 6 heads
  out    [2048, 768]  f32   partial projection output

Schedule note — the chip power limiter is the binding constraint: all 8 cores
share one NeuronDevice's power envelope, and firmware clamps the PE to K=4/8
(1.2 GHz) when sustained PE duty is too high. Variants that overlap qkv into
the attention flow (raising attention-phase PE duty from ~80% toward ~95%)
measured 373->477/412/373 us with 120-180 us spent clamped. The schedule here
(qkv burst upfront, then ScalarE-exp-paced attention at ~80% PE duty)
measures fastest (318 us, throttle ~13-23 us). Attention is ScalarE-paced:
exp of all 25.2M logits at 153.6 G/s is ~170 us min; PE stream is 225 us.

Pipeline:
  - PE warmup matmuls bridge the initial DMA window so the HAM clock gate is
    warm when the real qkv stream starts.
  - qkv projection in bf16 (psum f32), all upfront; q/k stored transposed
    [d, n] packed two heads per 128-partition tile; v stored [token, d]
    augmented with a ones column per head ([64 v | 1] x 6 heads).
  - scores: S^T chunks [128 kv-rows, 512 q] = kT-slice.T @ qT-slice (K=64);
    exp on ScalarE over [128, <=1536] PSUM spans with the 1/sqrt(Dh) scale
    folded into the ACTIVATE affine (no max-subtraction; |S|<=~10 here).
  - PV: one matmul per (kv-chunk, head) with augmented V stationary [128, 65]
    -> attn-out.T rows 0..63 + softmax denominator at row 64 of the same PSUM
    accumulation group.
  - the two heads of a pair run as separate score/exp/PV streams, so each
    head's normalize chain (DVE reciprocal + gpsimd partition-broadcast +
    DVE multiply -> bf16 attnT tiles) overlaps the other head's stream
    instead of stalling the next unit's PV accumulation.
  - projection of query block qb runs inside units (qb+1, hp0/hp1), 2 row
    blocks each: the unit emits its h2=0 scores+exps first, then the proj
    matmuls, then the deferred PVs, so ScalarE chews the queued exp backlog
    instead of idling while the PE projects. Last block's projection tails.
  PSUM: st-tag 2x3 banks (shared by qkv groups, scores, proj) + pv 2x1 = 8.
"""

import sys

if "/opt/trn_rl_repo" not in sys.path:
    sys.path.insert(0, "/opt/trn_rl_repo")

import numpy as np
import ml_dtypes

B, N, C = 4, 2048, 768
H, Dh = 12, 64
HC = H // 2            # heads per core
HPC = HC // 2          # head-pairs per core (3)
CW = HC * Dh           # 384 output-d columns per core
SCALE = Dh ** -0.5
CCH = C // 128         # 6 contraction chunks
NCORES = 8
JG = [(0, 3), (3, 3), (6, 3), (9, 3), (12, 2), (14, 2)]  # kv j-block groups

_NC_CACHE = {}


def _build():
    import concourse.bass as bass
    import concourse.tile as tile
    import concourse.mybir as mybir
    from concourse import bacc

    f32 = mybir.dt.float32
    bf16 = mybir.dt.bfloat16
    Exp = mybir.ActivationFunctionType.Exp

    nc = bacc.Bacc(
        "TRN2",
        target_bir_lowering=False,
        debug=False,
        enable_asserts=False,
        num_devices=NCORES,
    )

    xT = nc.dram_tensor("xT", [C, N], bf16, kind="ExternalInput").ap()
    wqkvT = nc.dram_tensor("wqkvT", [C, 3 * CW], bf16, kind="ExternalInput").ap()
    wprojT = nc.dram_tensor("wprojT", [CW, C], bf16, kind="ExternalInput").ap()
    out = nc.dram_tensor("out", [N, C], f32, kind="ExternalOutput").ap()

    with tile.TileContext(nc) as tc:
        from contextlib import ExitStack

        with ExitStack() as ctx:
            singles = ctx.enter_context(tc.tile_pool(name="singles", bufs=1))
            psum = ctx.enter_context(tc.tile_pool(name="psum", bufs=1, space="PSUM"))

            # ---- load phase-A inputs (released after qkv) ---------------
            load = tc.alloc_tile_pool(name="load", bufs=1)
            xt = [load.tile([128, N], bf16, tag=f"xt{i}", name=f"xt{i}")
                  for i in range(CCH)]
            wq = [load.tile([128, 3 * CW], bf16, tag=f"wq{i}", name=f"wq{i}")
                  for i in range(CCH)]
            for i in range(CCH):
                nc.gpsimd.dma_start(out=wq[i][:, 0:128],
                                    in_=wqkvT[i * 128:(i + 1) * 128, 0:128])
            for i in range(CCH):
                nc.gpsimd.dma_start(out=wq[i][:, 128:],
                                    in_=wqkvT[i * 128:(i + 1) * 128, 128:])
            for nch in range(N // 512):
                for i in range(CCH):
                    nc.sync.dma_start(
                        out=xt[i][:, nch * 512:(nch + 1) * 512],
                        in_=xT[i * 128:(i + 1) * 128, nch * 512:(nch + 1) * 512])
            wp = []
            for i in range(HPC):
                t = singles.tile([128, C], bf16, tag=f"wp{i}", name=f"wp{i}")
                nc.gpsimd.dma_start(out=t, in_=wprojT[i * 128:(i + 1) * 128, :])
                wp.append(t)

            # ---- PE warmup: HAM un-throttles after ~3.4 us of activity;
            # dummy matmuls on a memset tile (no DMA dependency) keep the
            # clock gate warm across the DMA window so the real qkv stream
            # starts at 2.4 GHz ------------------------------------------
            warm = singles.tile([128, 512], bf16, tag="warm", name="warm")
            nc.vector.memset(warm, 0.0)
            for _ in range(30):
                wps = psum.tile([128, 512], f32, tag="pv", bufs=2,
                                name="warmps")
                nc.tensor.matmul(wps, lhsT=warm[:, 0:128], rhs=warm,
                                 start=True, stop=True)

            # ---- qkv projections, results stored bf16 -------------------
            qt = [singles.tile([128, N], bf16, tag=f"qt{i}", name=f"qt{i}")
                  for i in range(HPC)]
            kt = [singles.tile([128, N], bf16, tag=f"kt{i}", name=f"kt{i}")
                  for i in range(HPC)]
            # v_aug: per 128-token tile, 6 heads x (64 v-cols + ones col)
            vt = [singles.tile([128, HC * 65], bf16, tag=f"vt{i}", name=f"vt{i}")
                  for i in range(N // 128)]

            # qT[d, n] and kT[d, n], two heads per 128-partition tile.
            # nch-outer so group order matches the xT chunk DMA arrival
            # order: the PE consumes ~7.7 us of groups per chunk while the
            # DMA delivers one every ~3.3 us, so it never stalls mid-phase
            # (a dt-major order stalled on chunk 3 and drew an HAM
            # re-throttle inside the qkv burst).
            for nch in range(N // 512):
                for dt in range(HPC):
                    for (dst, col0) in ((qt[dt], dt * 128),
                                        (kt[dt], CW + dt * 128)):
                        ps = psum.tile([128, 512], f32, tag="st", bufs=2,
                                       name="ps_qk")
                        for cc in range(CCH):
                            nc.tensor.matmul(
                                ps,
                                lhsT=wq[cc][:, col0:col0 + 128],
                                rhs=xt[cc][:, nch * 512:(nch + 1) * 512],
                                start=(cc == 0), stop=(cc == CCH - 1),
                            )
                        # ScalarE is idle all qkv phase and its copy is
                        # faster than DVE's, so the psum->sbuf drain never
                        # back-pressures the 2-buffer st rotation
                        nc.scalar.copy(
                            dst[:, nch * 512:(nch + 1) * 512], ps)

            # v in [token, d] layout: v[n, d] = sum_c xT[c, n] * wv[c, d]
            for nt in range(N // 128):
                vaug = vt[nt].rearrange("p (h e) -> p h e", e=65)
                nc.vector.memset(vaug[:, :, 64:65], 1.0)
                ps = psum.tile([128, 512], f32, tag="st", bufs=2, name="ps_v")
                for cc in range(CCH):
                    nc.tensor.matmul(
                        ps[:, :CW],
                        lhsT=xt[cc][:, nt * 128:(nt + 1) * 128],
                        rhs=wq[cc][:, 2 * CW:3 * CW],
                        start=(cc == 0), stop=(cc == CCH - 1),
                    )
                nc.vector.tensor_copy(
                    vaug[:, :, 0:64],
                    ps[:, :CW].rearrange("p (h e) -> p h e", e=64),
                )

            load.release()

            # ---- attention ----------------------------------------------
            work = ctx.enter_context(tc.tile_pool(name="work", bufs=4))
            att = [singles.tile([128, N], bf16, tag=f"att{i}", name=f"att{i}")
                   for i in range(HPC)]

            def proj_block(ic):
                """Projection for one 128-row block (partial: this core's d)."""
                pj = psum.tile([128, C], f32, tag="st", bufs=2, name="pj")
                # dt outer: the dt<2 matmuls depend only on already-normalized
                # head-pairs, so the tail blocks overlap the last unit's
                # normalize chain instead of queuing behind the dt=2 wait
                for dt in range(HPC):
                    for (d0, dw) in ((0, 512), (512, 256)):
                        nc.tensor.matmul(
                            pj[:, d0:d0 + dw],
                            lhsT=att[dt][:, ic * 128:(ic + 1) * 128],
                            rhs=wp[dt][:, d0:d0 + dw],
                            start=(dt == 0), stop=(dt == HPC - 1),
                        )
                osb = work.tile([128, C], f32, tag="osb", bufs=3, name="osb")
                nc.vector.tensor_copy(osb, pj)
                nc.sync.dma_start(out=out[ic * 128:(ic + 1) * 128, :], in_=osb)

            def attn_unit(qb, hp, proj_blocks=()):
                """Scores+exp+PV+normalize, one 512-q block x head pair.

                The two heads run as separate streams so each head's
                normalize chain overlaps the other head's stream. When
                proj_blocks is set (previous query block's projection), the
                h2=0 stream emits all scores+exps first, then the proj
                matmuls, then the deferred PVs — so ScalarE chews the queued
                exp backlog instead of idling while the PE projects.
                """
                pv = [psum.tile([128, 512], f32, tag="pv", bufs=2,
                                name=f"pv{h2}") for h2 in range(2)]
                for h2 in range(2):
                    hb = h2 * 64
                    defer = h2 == 0 and proj_blocks
                    pvq = []
                    for (j0, jn) in JG:          # j-groups of up to 3x128 rows
                        st = psum.tile([128, 1536], f32, tag="st", bufs=2,
                                       name="st")
                        for cx in range(jn):
                            j = j0 + cx
                            nc.tensor.matmul(
                                st[:, cx * 512:(cx + 1) * 512],
                                lhsT=kt[hp][hb:hb + 64, j * 128:(j + 1) * 128],
                                rhs=qt[hp][hb:hb + 64, qb * 512:(qb + 1) * 512],
                                start=True, stop=True,
                            )
                        et = work.tile([128, 1536], bf16, tag="et", bufs=8,
                                       name="et")
                        nc.scalar.activation(et[:, :jn * 512], st[:, :jn * 512],
                                             Exp, scale=SCALE)
                        pvq.append((et, j0, jn))
                        if not defer:
                            for (ete, pj0, pjn) in pvq:
                                for cx in range(pjn):
                                    j = pj0 + cx
                                    h = hp * 2 + h2
                                    nc.tensor.matmul(
                                        pv[h2][0:65, :],
                                        lhsT=vt[j][:, h * 65:(h + 1) * 65],
                                        rhs=ete[:, cx * 512:(cx + 1) * 512],
                                        start=(j == 0),
                                        stop=(j == N // 128 - 1),
                                    )
                            pvq = []
                    if defer:
                        for ic in proj_blocks:
                            proj_block(ic)
                        for (ete, pj0, pjn) in pvq:
                            for cx in range(pjn):
                                j = pj0 + cx
                                h = hp * 2
                                nc.tensor.matmul(
                                    pv[0][0:65, :],
                                    lhsT=vt[j][:, h * 65:(h + 1) * 65],
                                    rhs=ete[:, cx * 512:(cx + 1) * 512],
                                    start=(j == 0), stop=(j == N // 128 - 1),
                                )
                    srow = work.tile([1, 512], f32, tag="srow", bufs=4,
                                     name="srow")
                    nc.vector.tensor_copy(srow, pv[h2][64:65, :])
                    sinv = work.tile([1, 512], f32, tag="sinv", bufs=4,
                                     name="sinv")
                    nc.vector.reciprocal_approx_fast(sinv, srow)
                    bc = work.tile([64, 512], f32, tag="bc", bufs=4, name="bc")
                    nc.gpsimd.partition_broadcast(bc, sinv)
                    nc.vector.tensor_mul(
                        att[hp][h2 * 64:h2 * 64 + 64, qb * 512:(qb + 1) * 512],
                        pv[h2][0:64, :],
                        bc,
                    )

            for qb in range(N // 512):           # 512-wide query block
                for hp in range(HPC):            # head pair
                    if qb > 0 and hp < 2:
                        pblks = range((qb - 1) * 4 + 2 * hp,
                                      (qb - 1) * 4 + 2 * hp + 2)
                    else:
                        pblks = ()
                    attn_unit(qb, hp, proj_blocks=pblks)
            for ic in range(12, 16):
                proj_block(ic)

    nc.compile()
    return nc


def _get_nc():
    if "nc" not in _NC_CACHE:
        _NC_CACHE["nc"] = _build()
    return _NC_CACHE["nc"]


def _ensure_ntff_hook():
    """The agent image's ``antenv`` lacks ``axon_hooks``; synthesize it so
    ``run_bass_kernel_spmd(trace=True)`` can capture NTFF profiles."""
    import types
    try:
        from antenv.axon_hooks import get_axon_ntff_profile_hook  # noqa: F401
        return
    except ImportError:
        pass
    import antenv
    from trn_agent_boot.trn_boot import _ntff_profile_via_ctypes
    hook = _ntff_profile_via_ctypes("/opt/axon/libaxon_pjrt.so")
    mod = types.ModuleType("antenv.axon_hooks")
    mod._hook = hook
    mod.get_axon_ntff_profile_hook = lambda: mod._hook

    def _set(h):
        mod._hook = h

    mod.set_axon_ntff_profile_hook = _set
    sys.modules["antenv.axon_hooks"] = mod
    antenv.axon_hooks = mod


def kernel(trace=False, **inputs):
    x = np.asarray(inputs["x"], np.float32)
    qkv_w = np.asarray(inputs["qkv_w"], np.float32)
    proj_w = np.asarray(inputs["proj_w"], np.float32)
    proj_b = np.asarray(inputs["proj_b"], np.float32)

    nc = _get_nc()

    xTb = np.ascontiguousarray(x.transpose(0, 2, 1)).astype(ml_dtypes.bfloat16)
    wqkvT = np.ascontiguousarray(qkv_w.T).astype(ml_dtypes.bfloat16)
    wprojT = np.ascontiguousarray(proj_w.T).astype(ml_dtypes.bfloat16)

    in_maps = []
    for c in range(NCORES):
        b, hg = divmod(c, 2)
        cs = hg * CW
        wslice = np.concatenate(
            [wqkvT[:, s + cs:s + cs + CW] for s in (0, C, 2 * C)], axis=1)
        in_maps.append({
            "xT": xTb[b],
            "wqkvT": np.ascontiguousarray(wslice),
            "wprojT": np.ascontiguousarray(wprojT[cs:cs + CW, :]),
        })

    from concourse import bass_utils
    if trace:
        _ensure_ntff_hook()
        bass_utils.upload_artifacts = lambda tmpdir: tmpdir
    res = bass_utils.run_bass_kernel_spmd(
        nc, in_maps, core_ids=list(range(NCORES)), trace=trace,
    )

    out = np.empty((B, N, C), np.float32)
    for b in range(B):
        out[b] = res.results[2 * b]["out"] + res.results[2 * b + 1]["out"]
    out += proj_b

    if trace:
        return out, res
    return out
